# revision 42
# baseline (speedup 1.0000x reference)
"""Trainium2 Bass kernel for the 4-modality attention-fusion module.

Computes, for full inputs mod0..mod3 [16384, 1024] f32 and W [1024, 1024] f32:
    scores_m = mod_m @ W.T                      (per modality)
    attn     = softmax over m of scores         (elementwise over [B, L])
    fused    = sum_m mod_m * attn_m
    scaler_b = 1 + #{m : sum_l mod_m[b, l] == 0}
    out      = fused * scaler[:, None]

Sharded data-parallel over 8 NeuronCores along the batch dim (2048 rows each),
W replicated. Design (v3, measurement-driven):
  - the whole pipeline runs in bf16 except PSUM accumulation and the final
    output: HW-measured PE rate is 1 moving column/cycle regardless of
    fp32r/bf16, but bf16 halves transpose cost, SBUF footprint, and enables
    the DVE 2x mode in the tail; end-to-end error ~7.7e-3 vs the 2e-2 gate;
  - inputs arrive as bf16 via casting SWDGE DMAs (gpsimd) — measured free
    (+0.4us/tile); the fp32 mod tiles never exist in SBUF;
  - mod transposes run on the PE (bf16, 1 cyc/row) into one PSUM bank per
    mod, evicted by ACT in a single [128,1024] copy (the DMA-XBAR
    dma_start_transpose path measured ~3x slower than PE on HW);
  - mains: per tile 64 ldweights+matmul pairs (bf16 stationary+moving,
    fp32 PSUM), measured at the 218ns/512-col-pair array roofline;
  - ACT: 8 exps (PSUM -> bf16 SBUF) + 4 evictions; DVE: rowsums
    (tensor_scalar accum, 4x mode) + softmax tail in bf16 (2x mode), fp32
    only for reciprocal_approx_fast and the final scaled output;
  - zero-modality rescale folded into the final scalar_tensor_tensor;
  - 4-tile cast lead / 3-tile transpose lead; all ablations (exps, tail,
    rowsums) measured fully hidden under the PE stream.
"""

import sys

sys.path.insert(0, "/opt/trn_rl_repo")

from contextlib import ExitStack

import numpy as np

import concourse.bass as bass
import concourse.bacc as bacc
import concourse.mybir as mybir
import concourse.tile as tile
from concourse.bass_utils import run_bass_kernel_spmd
from concourse.masks import make_identity

F32 = mybir.dt.float32
BF16 = mybir.dt.bfloat16
AF = mybir.ActivationFunctionType

N_CORES = 8
B_FULL = 16384
L = 1024
P = 128
B_SHARD = B_FULL // N_CORES          # 2048
NPT = B_SHARD // P                   # 16 patient tiles per core
NM = 4                               # modalities
NLC = L // P                         # 8 l-chunks (contraction)
NH = 2                               # k halves
KH = L // NH                         # 512

_CACHE: dict = {}


def _build(
    repeat: int = 1,
    *,
    xpose: str = "pe",       # "pe" | "dma" | "dma8" | "off"
    cast: str = "swdge",     # "swdge" | "pool"
    den_engine: str = "dve", # "dve" | "pool"
    elem: bool = True,       # ablation: exps + tail
    rowsums: bool = True,    # ablation: zero-modality detection
    rs: str = "dve",         # rowsum engine: "act" | "dve"
    mains: str = "real",     # "real" | "pure" (no cross-engine deps) | "off"
    feed: str = "on",        # "on" | "off" (skip casts+transposes; needs pure)
    ptb: int = 2,            # PSUM banks for the transpose pool (sq gets 8-ptb)
    warm: int = 0,           # PE p-state warm-up dummy matmuls before W-prep
):
    nc = bacc.Bacc("TRN2", target_bir_lowering=False, debug=False)
    mods_d = [
        nc.dram_tensor(f"mod{m}", [B_SHARD, L], F32, kind="ExternalInput").ap()
        for m in range(NM)
    ]
    w_d = nc.dram_tensor("W", [L, L], F32, kind="ExternalInput").ap()
    out_d = nc.dram_tensor("out", [B_SHARD, L], F32, kind="ExternalOutput").ap()

    with tile.TileContext(nc) as tc, ExitStack() as ctx:
        const_p = ctx.enter_context(tc.tile_pool(name="const", bufs=1))
        wt_p = ctx.enter_context(tc.tile_pool(name="wt", bufs=1))
        wload_p = ctx.enter_context(tc.tile_pool(name="wload", bufs=2))
        mb_p = ctx.enter_context(tc.tile_pool(name="mb", bufs=6))
        mf_p = ctx.enter_context(tc.tile_pool(name="mf", bufs=2))
        mt_p = ctx.enter_context(tc.tile_pool(name="mt", bufs=5))
        e_p = ctx.enter_context(tc.tile_pool(name="e", bufs=4))
        tmp_p = ctx.enter_context(tc.tile_pool(name="tmp", bufs=2))
        out_p = ctx.enter_context(tc.tile_pool(name="outp", bufs=2))

        identW = const_p.tile([P, P], F32, tag="identW")
        make_identity(nc, identW[:])
        if xpose in ("pe", "mix"):
            ident = const_p.tile([P, P], BF16, tag="ident")
            make_identity(nc, ident[:])
        if warm:
            # PE p-state warm-up: ~3us of dependency-free array work so the
            # clock is at 2.4 GHz when the first real transposes/mains issue
            warm_src = const_p.tile([P, 4 * P], BF16, tag="warm_src")
            nc.vector.memset(warm_src[:], 0.0)

        # ---- Build wt resident in SBUF (bf16): wt[p, lc, k] = W[k, lc*128+p]
        # W loads ride the fast HWDGE path as fp32; the (otherwise idle) PE
        # transposes each block and the ACT eviction rounds to bf16. This
        # keeps the XBAR free for tile 0/1 mod transposes during startup.
        # The transpose PSUM pool is scoped so the main loop gets all 8 banks.
        wt = wt_p.tile([P, NLC, L], BF16, tag="wt")
        with tc.tile_pool(
            name="ps_w", bufs=2, space=bass.MemorySpace.PSUM
        ) as ps_w:
            if warm:
                warm_ps = ps_w.tile([P, 4 * P], F32, tag="warm_ps", bufs=1)
                for wmi in range(warm):
                    nc.tensor.matmul(
                        warm_ps[:],
                        warm_src[:, 0:P],
                        warm_src[:],
                        start=True,
                        stop=True,
                        skip_group_check=True,
                    )
            wks = []
            for kb in range(NLC):
                wk = wload_p.tile([P, L], F32, tag="wk", name=f"wk{kb}")
                nc.sync.dma_start(wk[:], w_d[kb * P : (kb + 1) * P, :])
                wks.append(wk)
            for kb in range(NLC):
                for g in range(2):
                    ptw = ps_w.tile(
                        [P, 4 * P], F32, tag="ptw", name=f"ptw{kb}g{g}"
                    )
                    for j in range(4):
                        lc = g * 4 + j
                        nc.tensor.transpose(
                            ptw[:, j * P : (j + 1) * P],
                            wks[kb][:, lc * P : (lc + 1) * P],
                            identW[:],
                        )
                    nc.scalar.copy(
                        wt[:, g * 4 : (g + 1) * 4, kb * P : (kb + 1) * P],
                        ptw[:],
                    )

        n_sq_bufs = (8 - ptb) if xpose in ("pe", "mix") else 8
        ps_q = ctx.enter_context(
            tc.tile_pool(name="ps_q", bufs=n_sq_bufs, space=bass.MemorySpace.PSUM)
        )
        if xpose in ("pe", "mix"):
            ps_t = ctx.enter_context(
                tc.tile_pool(name="ps_t", bufs=ptb, space=bass.MemorySpace.PSUM)
            )

        # ---------------- main loop, software-pipelined ----------------
        rep_cm = (
            tc.For_i(
                0,
                repeat,
                1,
                hint_engines=(
                    mybir.EngineType.PE,
                    mybir.EngineType.DVE,
                    mybir.EngineType.Activation,
                    mybir.EngineType.Pool,
                    mybir.EngineType.SP,
                ),
            )
            if repeat > 1
            else None
        )
        if rep_cm is not None:
            rep_cm.__enter__()

        def emit_cast(p):
            """Casting input DMAs: mod rows fp32 -> bf16 SBUF."""
            row = slice(p * P, (p + 1) * P)
            mbs = []
            for m in range(NM):
                mb = mb_p.tile([P, L], BF16, tag=f"mb{m}")
                if cast == "swdge":
                    nc.gpsimd.dma_start(mb[:], mods_d[m][row, :])
                else:
                    mf = mf_p.tile([P, L], F32, tag=f"mf{m}")
                    nc.sync.dma_start(mf[:], mods_d[m][row, :])
                    nc.gpsimd.tensor_copy(mb[:], mf[:])
                mbs.append(mb)
            return mbs

        def emit_xpose(p, mbs):
            """modT[p, lc, b] = mod[b, lc*128+p], via DMA XBAR or PE."""
            if xpose == "off":
                return None
            mts = []
            for m in range(NM):
                mt = mt_p.tile([P, NLC, P], BF16, tag=f"mt{m}")
                if xpose == "dma" or (xpose == "mix" and m >= 2):
                    nc.sync.dma_start_transpose(mt[:], mbs[m][:])
                elif xpose == "dma8":
                    for lc in range(NLC):
                        nc.sync.dma_start_transpose(
                            mt[:, lc, :], mbs[m][:, lc * P : (lc + 1) * P]
                        )
                else:  # pe
                    # a whole bf16 mod transposes into one PSUM bank: 8
                    # transposes, then a single [P, 1024] eviction
                    pt = ps_t.tile([P, NLC * P], BF16, tag="pt")
                    for lc in range(NLC):
                        nc.tensor.transpose(
                            pt[:, lc * P : (lc + 1) * P],
                            mbs[m][:, lc * P : (lc + 1) * P],
                            ident[:],
                        )
                    nc.scalar.copy(mt[:], pt[:])
                mts.append(mt)
            return mts

        def emit_compute(p, mbs, mts):
            """PE score matmuls + ACT exps + ACT rowsums for tile p."""
            es = []
            psums = tmp_p.tile([P, NM], F32, tag="psums")
            for m in range(NM):
                if mains == "off":
                    break
                sqs = [
                    ps_q.tile([P, KH], F32, tag="sq", name=f"sq{m}h{h}")
                    for h in range(NH)
                ]
                if mains == "real":
                    for lc in range(NLC):
                        for h in range(NH):
                            nc.tensor.matmul(
                                sqs[h][:],
                                mts[m][:, lc, :],
                                wt[:, lc, h * KH : (h + 1) * KH],
                                start=(lc == 0),
                                stop=(lc == NLC - 1),
                            )
                elif mains == "pure":
                    # same instruction stream, but stationaries come from the
                    # resident wt tile: no dependency on casts/transposes
                    for lc in range(NLC):
                        for h in range(NH):
                            nc.tensor.matmul(
                                sqs[h][:],
                                wt[:, lc, 0:P],
                                wt[:, lc, h * KH : (h + 1) * KH],
                                start=(lc == 0),
                                stop=(lc == NLC - 1),
                            )
                e = e_p.tile([P, L], BF16, tag=f"e{m}")
                if elem:
                    for h in range(NH):
                        nc.scalar.activation(
                            e[:, h * KH : (h + 1) * KH], sqs[h][:], AF.Exp
                        )
                es.append(e)
                if rowsums:
                    scratch = tmp_p.tile([P, L], BF16, tag="scratch")
                    if rs == "dve":
                        nc.vector.tensor_scalar(
                            out=scratch[:],
                            in0=mbs[m][:],
                            scalar1=1.0,
                            scalar2=None,
                            op0=mybir.AluOpType.mult,
                            op1=mybir.AluOpType.add,
                            accum_out=psums[:, m : m + 1],
                        )
                    else:
                        nc.scalar.activation(
                            scratch[:], mbs[m][:], AF.Copy,
                            accum_out=psums[:, m : m + 1],
                        )
            return p, mbs, es, psums

        def emit_tail(state):
            """Softmax tail for tile p (lags one segment)."""
            p, mbs, es, psums = state
            row = slice(p * P, (p + 1) * P)
            if not elem:
                # ablation: write anything so the output DMA still runs
                ot = out_p.tile([P, L], F32, tag="ot")
                nc.vector.memset(ot[:], 0.0)
                nc.sync.dma_start(out_d[row, :], ot[:])
                return
            if not rowsums:
                nc.vector.memset(psums[:], 1.0)
            e0, e1, e2, e3 = es

            zt = tmp_p.tile([P, NM], F32, tag="zt")
            zs = tmp_p.tile([P, 1], F32, tag="zs")
            nc.vector.tensor_scalar(
                out=zt[:],
                in0=psums[:],
                scalar1=0.0,
                scalar2=None,
                op0=mybir.AluOpType.is_equal,
                op1=mybir.AluOpType.add,
                accum_out=zs[:],
            )
            scaler = tmp_p.tile([P, 1], F32, tag="scaler")
            nc.vector.tensor_scalar_add(scaler[:], zs[:], 1.0)

            adde = nc.gpsimd if den_engine == "pool" else nc.vector
            d01 = tmp_p.tile([P, L], BF16, tag="d01")
            d23 = tmp_p.tile([P, L], BF16, tag="d23")
            den = tmp_p.tile([P, L], F32, tag="den")
            adde.tensor_add(d01[:], e0[:], e1[:])
            adde.tensor_add(d23[:], e2[:], e3[:])
            adde.tensor_add(den[:], d01[:], d23[:])
            # numerator: e_m *= mod_m in place (den reads already queued)
            for m in range(NM):
                nc.vector.tensor_mul(es[m][:], es[m][:], mbs[m][:])
            nc.vector.tensor_add(e0[:], e0[:], e1[:])
            nc.vector.tensor_add(e2[:], e2[:], e3[:])
            nc.vector.tensor_add(e0[:], e0[:], e2[:])
            r = tmp_p.tile([P, L], F32, tag="r")
            nc.vector.reciprocal_approx_fast(out=r[:], in_=den[:])
            ot = out_p.tile([P, L], F32, tag="ot")
            nc.vector.scalar_tensor_tensor(
                out=ot[:],
                in0=r[:],
                scalar=scaler[:],
                in1=e0[:],
                op0=mybir.AluOpType.mult,
                op1=mybir.AluOpType.mult,
            )
            nc.sync.dma_start(out_d[row, :], ot[:])

        # 3-tile cast lead and 2-tile transpose lead: mains(p) must never
        # wait on the mT transposes or the input casts
        if feed == "off":
            assert mains != "real"
            prev = None
            for p in range(NPT):
                state = emit_compute(p, None, None)
                if prev is not None:
                    emit_tail(prev)
                prev = state
            emit_tail(prev)
        else:
            mbs_by_p = {}
            mts_by_p = {}
            for q in range(4):
                mbs_by_p[q] = emit_cast(q)
            for q in range(3):
                mts_by_p[q] = emit_xpose(q, mbs_by_p[q])
            prev = None
            for p in range(NPT):
                if p + 3 < NPT:
                    mts_by_p[p + 3] = emit_xpose(p + 3, mbs_by_p[p + 3])
                state = emit_compute(p, mbs_by_p[p], mts_by_p[p])
                if prev is not None:
                    emit_tail(prev)
                if p + 4 < NPT:
                    mbs_by_p[p + 4] = emit_cast(p + 4)
                prev = state
            emit_tail(prev)

        if rep_cm is not None:
            rep_cm.__exit__(None, None, None)

    nc.compile()
    return nc


def _get_nc(repeat: int = 1, **flags):
    key = ("nc", repeat, tuple(sorted(flags.items())))
    if key not in _CACHE:
        _CACHE[key] = _build(repeat, **flags)
    return _CACHE[key]


def _run(inputs, trace=False):
    nc = _get_nc()
    w = np.ascontiguousarray(np.asarray(inputs["W"], dtype=np.float32))
    in_maps = []
    for c in range(N_CORES):
        sl = slice(c * B_SHARD, (c + 1) * B_SHARD)
        im = {"W": w}
        for m in range(NM):
            im[f"mod{m}"] = np.ascontiguousarray(
                np.asarray(inputs[f"mod{m}"], dtype=np.float32)[sl]
            )
        in_maps.append(im)
    return run_bass_kernel_spmd(
        nc, in_maps, core_ids=list(range(N_CORES)), trace=trace
    )


def kernel(**inputs) -> np.ndarray:
    res = _run(inputs, trace=False)
    return np.concatenate(
        [res.results[c]["out"] for c in range(N_CORES)], axis=0
    ).astype(np.float32)
